# revision 8
# baseline (speedup 1.0000x reference)
"""Trainium2 Bass kernel for the ExportableStudentSNN1d problem.

Data-parallel over batch: 64 samples -> 8 cores x 8 samples. Each core runs
an identical NEFF on its batch shard; host concatenates the [8, 4] outputs.

Math notes (TAU1 = 1.0 makes layer-1 LIF memoryless):
  s1_t = (conv1(x_t)*G + b1*G >= TH1)        <=> conv1(x_t) >= TH1/G - b1
  layer2: v2pre = (10/9)*a2 - (1/9)*v2_prev,  a2 = G*(conv2(s1_t) + b2)
     The whole layer-2 recurrence is run in per-channel SCALED units: channel
     c2 is multiplied by s_c = 224/max|W2[c2]*10/3| so the fp8 e4m3 weights
     use the full exponent range (no subnormals).  Thresholds/biases scale
     along: th2_c = s_c*TH2, b2p_c = s_c*(10/3)*b2_c - 0.5*sum(qerr_c)
     (the 0.5 is the mean s1 activity; removes the quantization-bias term).
     y      = (psum2 + b2p) + carry = s_c*v2pre
     carry' = min(y, th2_c) * (-1/9)  (exact when no spike; spiked positions
              get -th2_c/9 instead of 0 -- ~2e-3*s_c perturbation, negligible)
     spike counts via ACT Sign(y - th2_c) accumulation: count = (sum_sign+n)/2,
     folded into the FC scale/bias on host (signs are scale-invariant).
  out[b,c] = (sum_{t,l} sp)/(T*L) @ Wfc.T + bfc

Layout: host pre-transposes x to [B, Cin, T, L] and casts to bf16 so the
conv rhs reads are unit-stride; conv1 runs in bf16 (fp32 PSUM accum).
conv1's im2col is materialized by DMA: 9 tap-shifted replicas of x across
108 SBUF partitions, so conv1 is a single K=108 matmul per 512-column
chunk (vs 9 accumulating K=12 matmuls).

conv2 runs in fp8 e4m3 with perf_mode=DoubleRow: s1 is binary so fp8 is
exact for the moving operand, and taps are paired (2j, 2j+1) so each
DoubleRow matmul contracts 2 taps x 128 channels in one PE pass (2 MACs
per cell per cycle). 9 taps = 4 DoubleRow MMs + 1 normal fp8 MM per
512-column chunk, ~1.8x fewer PE cycles than the bf16 9-tap version.
"""

import numpy as np
import ml_dtypes

import concourse.bacc as bacc
import concourse.tile as tile
import concourse.mybir as mybir
from concourse.ap import AP
from concourse.bass_utils import run_bass_kernel_spmd

F32 = mybir.dt.float32
BF16 = mybir.dt.bfloat16
F8 = mybir.dt.float8e4

N_CORES = 8
B, C_IN, L, T = 64, 12, 2048, 20
C1, C2, K, PAD = 128, 256, 9, 4
GAIN, TAU2, TH1, TH2 = 3.0, 0.9, 0.02, 0.02
NCLS = 4
B_SH = B // N_CORES            # 8 samples per core
LH = 1024                      # L processed in halves
HALO = 8                       # x halo per side (conv1 then conv2 shifts)
S1W = LH + 2 * PAD             # 1032 s1 columns needed per L-half
XW = LH + 2 * HALO             # 1040 x columns staged per L-half
A2S = (10.0 / 9.0) * GAIN      # 10/3: multiplier on conv2 psum
MDECAY = 1.0 / 9.0

_CACHE = {}


def _pair_rhs(s1, c0):
    """[128, 2, 512] moving-operand view pairing taps (c0, c0+1): element
    (p, i, j) = s1[p, c0 + i + j]. Overlapping strides are legal for the
    moving AP (reads only)."""
    base = s1[:, c0 : c0 + 512]
    return AP(
        tensor=base.tensor,
        offset=base.offset,
        ap=[list(base.ap[0]), [1, 2], [1, 512]],
    )


def _build():
    nc = bacc.Bacc("TRN2", target_bir_lowering=False, debug=False)

    x_d = nc.dram_tensor("x", [B_SH, C_IN, T, L], BF16, kind="ExternalInput")
    w1t_d = nc.dram_tensor("w1t", [K * C_IN, C1], BF16, kind="ExternalInput")
    w2t_d = nc.dram_tensor("w2t", [C1, K * C2], F8, kind="ExternalInput")
    th1_d = nc.dram_tensor("th1", [C1, 1], F32, kind="ExternalInput")
    b2p_d = nc.dram_tensor("b2p", [C1, 2], F32, kind="ExternalInput")
    th2_d = nc.dram_tensor("th2", [C1, 2], F32, kind="ExternalInput")
    nth2_d = nc.dram_tensor("nth2", [C1, 2], F32, kind="ExternalInput")
    wfc_d = nc.dram_tensor("wfc", [C1, 2 * NCLS], F32, kind="ExternalInput")
    bfc_d = nc.dram_tensor("bfc", [NCLS, 1], F32, kind="ExternalInput")
    out_d = nc.dram_tensor("out", [B_SH, NCLS], F32, kind="ExternalOutput")

    with tile.TileContext(nc) as tc:
        with (
            tc.tile_pool(name="const", bufs=1) as cpool,
            tc.tile_pool(name="xstage", bufs=2) as xpool,
            tc.tile_pool(name="s1", bufs=2) as s1pool,
            tc.tile_pool(name="lif", bufs=3) as lifpool,
            tc.tile_pool(name="carry", bufs=2) as cpool2,
            tc.tile_pool(name="psum1", bufs=1, space="PSUM") as pp1,
            tc.tile_pool(name="psum2", bufs=2, space="PSUM") as pp2,
            tc.tile_pool(name="psfc", bufs=1, space="PSUM") as ppfc,
        ):
            # ---- constants / weights (resident) ----
            # w1t rows (12k+ci) hold W1[:, ci, k] (im2col layout)
            w1t = cpool.tile([K * C_IN, C1], BF16)
            nc.sync.dma_start(w1t[:], w1t_d.ap())
            w2t = cpool.tile([C1, K * C2], F8)
            nc.sync.dma_start(w2t[:], w2t_d.ap())
            w2v = w2t[:].rearrange("p (k c) -> p k c", c=C2)
            th1 = cpool.tile([C1, 1], F32)
            nc.sync.dma_start(th1[:], th1_d.ap())
            b2p = cpool.tile([C1, 2], F32)
            nc.sync.dma_start(b2p[:], b2p_d.ap())
            th2 = cpool.tile([C1, 2], F32)
            nc.sync.dma_start(th2[:], th2_d.ap())
            nth2 = cpool.tile([C1, 2], F32)
            nc.sync.dma_start(nth2[:], nth2_d.ap())
            wfc = cpool.tile([C1, 2 * NCLS], F32)
            nc.sync.dma_start(wfc[:], wfc_d.ap())
            bfc = cpool.tile([NCLS, 1], F32)
            nc.sync.dma_start(bfc[:], bfc_d.ap())
            # spike counts, one column per (h, b, lh, t)
            acc = cpool.tile([C1, 2 * B_SH * 2 * T], F32)

            segs = [(b, lh) for b in range(B_SH) for lh in range(2)]

            def stage_segment(idx):
                # im2col staging: rows (12k+ci) = x[ci] shifted by tap k.
                # column (t, c) of row-group k = x[b, ci, t, l0+c+k-8]
                b, lh = segs[idx]
                l0 = lh * LH
                xs = xpool.tile([K * C_IN, T * S1W], BF16)
                xsv = xs[:].rearrange("p (t c) -> p t c", c=S1W)
                # zero the possible halo bands (32-aligned base partition
                # required for engine ops -> memset all rows; the DMAs
                # below overwrite whatever is valid)
                if l0 == 0:
                    nc.gpsimd.memset(xsv[:, :, 0:HALO], 0.0)
                if l0 + LH == L:
                    nc.gpsimd.memset(xsv[:, :, S1W - HALO : S1W], 0.0)
                for k in range(K):
                    rows = slice(C_IN * k, C_IN * (k + 1))
                    c_lo = max(0, HALO - k - l0)
                    c_hi = min(S1W, L - l0 - k + HALO)
                    src = x_d.ap()[b, :, :,
                                   l0 + c_lo + k - HALO : l0 + c_hi + k - HALO]
                    if idx == 0:
                        # cold start: split so the first timesteps' columns
                        # land first
                        nc.sync.dma_start(
                            xsv[rows, 0:2, c_lo:c_hi], src[:, 0:2, :])
                        nc.sync.dma_start(
                            xsv[rows, 2:T, c_lo:c_hi], src[:, 2:T, :])
                    else:
                        nc.sync.dma_start(xsv[rows, :, c_lo:c_hi], src)
                carry = cpool2.tile([C1, 2 * LH], F32)
                nc.gpsimd.memset(carry[:], 0.0)
                return xs, carry

            def conv1_block(xs, t):
                # conv1: one K=108 matmul per chunk, then s1 (fp8) on DVE
                p1 = pp1.tile([C1, 1536], F32)
                for c0, cn in ((0, 512), (512, 512), (1024, S1W - 1024)):
                    nc.tensor.matmul(
                        p1[:, c0 : c0 + cn],
                        w1t[:],
                        xs[:, t * S1W + c0 : t * S1W + c0 + cn],
                        start=True,
                        stop=True,
                    )
                s1 = s1pool.tile([C1, S1W], F8)
                nc.vector.tensor_scalar(
                    s1[:], p1[:, 0:S1W], th1[:], None,
                    op0=mybir.AluOpType.is_ge,
                )
                return s1

            def conv2_block(s1, h):
                # fp8 DoubleRow: taps (0,1),(2,3),(4,5),(6,7) fused pairwise,
                # tap 8 as a normal fp8 matmul. 5 PE passes per 512-col chunk.
                p2 = pp2.tile([C1, LH], F32)
                for c0 in (0, 512):
                    for j in range(4):
                        nc.tensor.matmul(
                            p2[:, c0 : c0 + 512],
                            w2v[:, 2 * j : 2 * j + 2, h * C1 : h * C1 + C1],
                            _pair_rhs(s1, c0 + 2 * j),
                            start=(j == 0),
                            stop=False,
                            perf_mode=mybir.MatmulPerfMode.DoubleRow,
                        )
                    nc.tensor.matmul(
                        p2[:, c0 : c0 + 512],
                        w2v[:, 8, h * C1 : h * C1 + C1],
                        s1[:, c0 + 8 : c0 + 8 + 512],
                        start=False,
                        stop=True,
                    )
                return p2

            def lif_front(carry, p2, h):
                # y = (psum2 + b2p) + carry
                ch = carry[:, h * LH : (h + 1) * LH]
                y = lifpool.tile([C1, LH], F32, tag="y")
                nc.vector.scalar_tensor_tensor(
                    y[:], p2[:], b2p[:, h : h + 1], ch,
                    op0=mybir.AluOpType.add, op1=mybir.AluOpType.add,
                )
                return y

            def lif_back(carry, y, h, col):
                # sign-sum for spike counting (off the carry chain)
                sg = lifpool.tile([C1, LH], F32, tag="sg")
                nc.scalar.activation(
                    sg[:], y[:], mybir.ActivationFunctionType.Sign,
                    bias=nth2[:, h : h + 1],
                    accum_out=acc[:, col + h * (B_SH * 2 * T) :
                                  col + h * (B_SH * 2 * T) + 1],
                )
                # carry' = min(y, th2) * (-1/9): exact for y < th2; spiked
                # positions get -th2/9 (vs 0) -- keeps the chain on DVE
                ch = carry[:, h * LH : (h + 1) * LH]
                nc.vector.tensor_scalar(
                    ch, y[:], th2[:, h : h + 1], -MDECAY,
                    op0=mybir.AluOpType.min, op1=mybir.AluOpType.mult,
                )

            # conv1 of segment idx+1's t=0 fills the empty t=19 pipeline
            # slot of segment idx, so segment boundaries don't stall PE
            staged = stage_segment(0)
            s1_cur = conv1_block(staged[0], 0)
            for idx in range(len(segs)):
                b, lh = segs[idx]
                xs, carry = staged
                if idx + 1 < len(segs):
                    staged = stage_segment(idx + 1)
                for t in range(T):
                    col = b * (2 * T) + lh * T + t
                    p2_0 = conv2_block(s1_cur, 0)
                    # emit conv1(t+1)+s1(t+1) before the h0 LIF ops: PE order
                    # is unchanged, but s1 lands earlier on DVE so
                    # conv2(t+1,h0) never waits on it
                    if t + 1 < T:
                        s1_next = conv1_block(xs, t + 1)
                    elif idx + 1 < len(segs):
                        s1_next = conv1_block(staged[0], 0)
                    else:
                        s1_next = None
                    y0 = lif_front(carry, p2_0, 0)
                    lif_back(carry, y0, 0, col)
                    p2_1 = conv2_block(s1_cur, 1)
                    y1 = lif_front(carry, p2_1, 1)
                    lif_back(carry, y1, 1, col)
                    s1_cur = s1_next

            # ---- pooling + FC head ----
            pooled = cpool.tile([C1, 2 * B_SH], F32)
            nc.vector.tensor_reduce(
                pooled[:],
                acc[:].rearrange("p (h b c) -> p (h b) c", h=2, b=B_SH),
                axis=mybir.AxisListType.X, op=mybir.AluOpType.add,
            )
            pfc = ppfc.tile([NCLS, B_SH], F32)
            for h in range(2):
                nc.tensor.matmul(
                    pfc[:],
                    wfc[:, h * NCLS : (h + 1) * NCLS],
                    pooled[:, h * B_SH : (h + 1) * B_SH],
                    start=(h == 0),
                    stop=(h == 1),
                )
            # pfc holds Wfc @ sign_sums; counts = (sign_sum + T*L)/2 is folded
            # into scale and the host-adjusted bias
            fin = cpool.tile([NCLS, B_SH], F32)
            nc.scalar.activation(
                fin[:], pfc[:], mybir.ActivationFunctionType.Identity,
                bias=bfc[:], scale=1.0 / float(2 * T * L),
            )
            nc.sync.dma_start(out_d.ap().rearrange("b c -> c b"), fin[:])

    nc.compile()
    return nc


def _prep_consts(W1, b1, W2, b2, Wfc, bfc):
    # w1t im2col layout: row (12k+ci), col co = W1[co, ci, k]
    w1t = np.ascontiguousarray(W1.transpose(2, 1, 0)).reshape(K * C_IN, C1)
    # Per-output-channel scaled fp8 e4m3 weights: channel c2 scaled so its
    # max |weight| sits at 224 (TRN e4m3 max normal is 240) -- no subnormals.
    sc = (224.0 / np.abs(W2 * A2S).max(axis=(1, 2))).astype(np.float32)  # [C2]
    w2s = (W2 * A2S * sc[:, None, None]).transpose(1, 2, 0)  # [C1, K, C2] f32
    w2q = w2s.astype(ml_dtypes.float8_e4m3)
    # fold the mean fp8 quantization error into the bias: s1 is ~Bernoulli(p)
    # per input channel; use p=0.5 (empirical activity is near half)
    qerr = w2q.astype(np.float32) - w2s               # [C1, K, C2]
    comp = -0.5 * qerr.sum(axis=(0, 1))               # [C2]
    w2t = np.ascontiguousarray(w2q).reshape(C1, K * C2)
    th1 = (TH1 / GAIN - b1).reshape(C1, 1).astype(np.float32)
    b2p_full = (A2S * b2 * sc + comp).astype(np.float32)
    b2p = b2p_full.reshape(2, C1).T.copy()            # [128, 2] cols = halves
    th2v = (TH2 * sc).astype(np.float32)
    th2 = th2v.reshape(2, C1).T.copy()                # [128, 2] cols = halves
    nth2 = (-th2v).reshape(2, C1).T.copy()
    wfcT = Wfc.T.reshape(2, C1, NCLS)                 # [2, 128, 4]
    wfc_t = wfcT.transpose(1, 0, 2).reshape(C1, 2 * NCLS).copy()
    # counts = (sign_sum + T*L)/2 folded into the FC epilogue:
    # out = (Wfc @ sign_sum)/(2*T*L) + (bfc + 0.5*rowsum(Wfc))
    bfc_c = (bfc + 0.5 * Wfc.sum(axis=1)).reshape(NCLS, 1).astype(np.float32)
    return {
        "w1t": w1t.astype(ml_dtypes.bfloat16),
        "w2t": w2t,
        "th1": th1,
        "b2p": b2p,
        "th2": th2,
        "nth2": nth2,
        "wfc": wfc_t.astype(np.float32),
        "bfc": bfc_c,
    }


def kernel(x, W1, b1, W2, b2, Wfc, bfc, _trace=False):
    x = np.asarray(x, dtype=np.float32)
    # [B, Cin, L, T] -> [B, Cin, T, L] bf16 so on-chip reads are unit-stride
    x_t = np.ascontiguousarray(x.transpose(0, 1, 3, 2)).astype(ml_dtypes.bfloat16)
    consts = _prep_consts(
        np.asarray(W1, np.float32), np.asarray(b1, np.float32),
        np.asarray(W2, np.float32), np.asarray(b2, np.float32),
        np.asarray(Wfc, np.float32), np.asarray(bfc, np.float32),
    )
    if "nc" not in _CACHE:
        _CACHE["nc"] = _build()
    nc = _CACHE["nc"]

    in_maps = []
    for c in range(N_CORES):
        m = dict(consts)
        m["x"] = np.ascontiguousarray(x_t[c * B_SH : (c + 1) * B_SH])
        in_maps.append(m)

    res = run_bass_kernel_spmd(
        nc, in_maps, core_ids=list(range(N_CORES)), trace=_trace
    )
    out = np.concatenate([res.results[c]["out"] for c in range(N_CORES)], axis=0)
    out = out.astype(np.float32)
    if _trace:
        return out, res
    return out
